# revision 13
# baseline (speedup 1.0000x reference)
"""AttentionMemory kernel for Trainium2 (8 NeuronCores, Bass/Tile).

Reference computation (per batch b):
    affinity[n, m] = (2 * mk[:,n]@qk[:,m] - ||mk[:,n]||^2 - ||qk[:,m]||^2) / 8
    out[n, m]      = softmax over n (memory axis)

Softmax over n is invariant to per-column (m) constants, so the
-||qk_m||^2 term and any global constant are dropped.  Logits come from
one augmented fp32r matmul:
    lhsT (stationary) = [0.25 * qk ; -0.125 ; -0.125]   -> [66, Mc]
    rhs  (moving)     = [mk        ; a_hi   ; a_lo  ]   -> [66, N]
    psum[m, n] = 0.25*dot(qk_m, mk_n) - 0.125*a'_n  == logits[m, n]
with a'_n = sum_c mk[c,n]^2 - mean(...) (centering keeps |a'| small so
fp32r rounding of the a-term is negligible; a is additionally split
hi/lo over two rows so the hi part is exact in reduced precision).
fp32r runs at bf16 speed on TRN2 when the moving free dim >= 256
(1 cycle/row); measured logit error ~1.4e-3 -> ~0.14% on the softmax.

The device ships softmax NUMERATORS (exp(logits), bf16) plus per-row
DENOMINATORS (f32 sums); the divide rides the host-side gather pass
that already casts/transposes the result.  bf16 numerator rounding is
a ~0.4% worst-case element error, ~5x under the 2e-2 gate, and halves
HBM store traffic vs f32: per-core 16.25 MB at the modeled 360 GB/s
aggregate DMA = 45 us.

Engine budget per core (cost model): ACT exp is the bottleneck
(1 elem/cycle/partition @ 1.2 GHz): 64512 cols * 0.83 ns + ~185 ns
per-call overhead ~= 60 us busy, gap-free after the first strip.  Row
sums ride DVE tensor_scalar 4x copies (accum_out, 0.26 ns/col) into a
scratch tile, avoiding ACT accum reads (187 ns/call).  PE fp32r ~29 us,
DVE ~22 us, DMA ~50 us — all under ACT.  Output stores depend only on
the exp, so they stream during the strip; the endgame is just the last
strip's sum chain (~3.5 us).

Strip 0 runs in [1,1,2,4]-chunk pieces so the ACT stream starts as
soon as the first 504-column m-chunk lands; the last strip's second
half runs as two 2-chunk pieces to shorten the drain.

Sharding: core c handles batch c//2, query-column half c%2 (softmax is
over the full n axis which each core holds).  Each core writes
out_c[m, n] bf16 + sums[126, 16]; the host divides, casts to f32 and
transposes to the reference [n, m] layout.
"""

import numpy as np

B, CK, H, W = 4, 64, 48, 84
N = H * W            # 4032 memory pixels (softmax axis)
HALF = N // 2        # 2016 query pixels per core
M_STRIP = 126        # output-partition strip size (16 * 126 = 2016)
N_STRIPS = HALF // M_STRIP
K_AUG = CK + 2       # 66: contraction dim incl. the a_hi / a_lo rows

N_CHUNK = 504        # matmul moving free dim (one PSUM bank, 8 pad cols)
N_CHUNKS = N // N_CHUNK  # 8

_CACHE = {}

# Input load schedule: (ring, tensor, col0, width).  Order = program order
# per ring; the single DMA wire serves transfers in ready order.
INPUT_PIECES = [
    ("pool", "m", 0, 504),
    ("sp", "q", 0, 252),
    ("pool", "m", 1008, 1008),
    ("sp", "m", 504, 504),
    ("pool", "m", 2016, 1008),
    ("sp", "m", 3024, 1008),
    ("sp", "q", 252, 1764),
]
# ACT piece widths (in 504-col chunks) per strip
PIECES_FIRST = [1, 1, 2, 2, 2]
PIECES_LAST = [4, 2, 1, 1]
PIECES_MID = [4, 4]
STORE_RING = "pool"      # steady-state store ring
STORE_RINGS_LAST = ["pool", "pool", "sp", "sp"]  # last strip, per piece
N_WARM = 12              # PE pstate spin-up matmuls
WARM_MEMSET_RING = "vector"  # engine for the warm-source memset


def _build_nc():
    import concourse.bacc as bacc
    import concourse.mybir as mybir
    import concourse.tile as tile

    f32 = mybir.dt.float32
    f32r = mybir.dt.float32r
    bf16 = mybir.dt.bfloat16
    Exp = mybir.ActivationFunctionType.Exp
    mult = mybir.AluOpType.mult
    add = mybir.AluOpType.add

    nc = bacc.Bacc("TRN2", target_bir_lowering=False, debug=False)

    q_d = nc.dram_tensor("q", [K_AUG, HALF], f32r, kind="ExternalInput")
    m_d = nc.dram_tensor("m", [K_AUG, N], f32r, kind="ExternalInput")
    out_d = nc.dram_tensor("out_c", [HALF, N], bf16, kind="ExternalOutput")
    sums_d = nc.dram_tensor(
        "sums", [M_STRIP, N_STRIPS], f32, kind="ExternalOutput"
    )

    with tile.TileContext(nc) as tc:
        with (
            tc.tile_pool(name="singles", bufs=1) as singles,
            tc.tile_pool(name="psum", bufs=2, space="PSUM") as psum_pool,
            tc.tile_pool(name="exp", bufs=3) as exp_pool,
            tc.tile_pool(name="scratch", bufs=2) as scr_pool,
            tc.tile_pool(name="stats", bufs=4) as stats_pool,
        ):
            # --- inputs, all on the SP ring in exact first-use order (one
            # shared DMA wire; a big transfer issued on another ring would
            # jump ahead of the m chunks and stall the ACT stream) --------
            q_s = singles.tile([K_AUG, HALF], f32r)
            m_s = singles.tile([K_AUG, N], f32r)
            sums_all = singles.tile([M_STRIP, N_STRIPS], f32)
            rings = {
                "sp": nc.sync,
                "act": nc.scalar,
                "pool": nc.gpsimd,
            }
            for ring, tens, c0, w in INPUT_PIECES:
                src_d, dst_s = (q_d, q_s) if tens == "q" else (m_d, m_s)
                rings[ring].dma_start(
                    out=dst_s[:, c0 : c0 + w], in_=src_d[:, c0 : c0 + w]
                )

            # --- prewarm: ACT exp table load + PE pstate spin-up during
            # the input DMAs ---------------------------------------------
            wtab = singles.tile([1, 2], f32)
            nc.vector.memset(wtab, 0.0)
            nc.scalar.activation(wtab[:, 1:2], wtab[:, 0:1], Exp)
            wsrc = singles.tile([K_AUG, 256], bf16)
            getattr(nc, WARM_MEMSET_RING).memset(wsrc, 0.0)
            wps = psum_pool.tile([M_STRIP, 256], f32, tag="ps")
            for _ in range(N_WARM):
                nc.tensor.matmul(
                    wps, wsrc[:, :M_STRIP], wsrc, start=True, stop=True
                )

            for s in range(N_STRIPS):
                m0 = s * M_STRIP
                q_l = q_s[:, m0 : m0 + M_STRIP]

                # piece widths in 504-col chunks; sum to 8 per strip
                if s == 0:
                    pieces = PIECES_FIRST
                elif s == N_STRIPS - 1:
                    pieces = PIECES_LAST
                else:
                    pieces = PIECES_MID

                exp_t = exp_pool.tile([M_STRIP, N], bf16, tag="exp")
                acc = stats_pool.tile([M_STRIP, len(pieces)], f32, tag="acc")

                c = 0
                for pi, w in enumerate(pieces):
                    ps = psum_pool.tile([M_STRIP, 512 * w], f32, tag="ps")
                    for cc in range(w):
                        nc.tensor.matmul(
                            ps[:, cc * 512 : cc * 512 + N_CHUNK],
                            q_l,
                            m_s[:, (c + cc) * N_CHUNK : (c + cc + 1) * N_CHUNK],
                            start=True,
                            stop=True,
                        )
                    # exp(logits) PSUM->SBUF bf16; 3D views skip the pad
                    e0 = c * N_CHUNK
                    e1 = (c + w) * N_CHUNK
                    nc.scalar.activation(
                        exp_t[:, e0:e1].rearrange("p (b c) -> p b c", b=w),
                        ps.rearrange("p (b c) -> p b c", b=w)[:, :, :N_CHUNK],
                        Exp,
                    )
                    # store the numerators as soon as the exp lands
                    # (Pool SWDGE ring in steady state: Pool is otherwise
                    # idle so store dispatch never contends; the drain-
                    # critical last strip uses the lowest-latency ring)
                    if s == N_STRIPS - 1:
                        sring = STORE_RINGS_LAST[pi]
                    else:
                        sring = STORE_RING
                    rings[sring].dma_start(
                        out=out_d[m0 : m0 + M_STRIP, e0:e1], in_=exp_t[:, e0:e1]
                    )
                    # row-sum rides a DVE 4x copy (accum_out) into scratch
                    scr = scr_pool.tile([M_STRIP, 4 * N_CHUNK], bf16, tag="scr")
                    nc.vector.tensor_scalar(
                        out=scr[:, : e1 - e0],
                        in0=exp_t[:, e0:e1],
                        scalar1=1.0,
                        scalar2=None,
                        op0=mult,
                        op1=add,
                        accum_out=acc[:, pi : pi + 1],
                    )
                    c += w

                nc.vector.reduce_sum(
                    sums_all[:, s : s + 1],
                    acc[:, : len(pieces)],
                    axis=mybir.AxisListType.X,
                )

            nc.scalar.dma_start(out=sums_d[:, :], in_=sums_all)

    nc.compile()
    return nc


def _get_nc():
    if "nc" not in _CACHE:
        _CACHE["nc"] = _build_nc()
    return _CACHE["nc"]


def kernel(mk: np.ndarray, qk: np.ndarray) -> np.ndarray:
    import ml_dtypes
    from concourse import bass_utils

    mk = np.asarray(mk, dtype=np.float32).reshape(B, CK, N)
    qk = np.asarray(qk, dtype=np.float32).reshape(B, CK, N)
    a = np.einsum("bcn,bcn->bn", mk, mk)      # sum_c mk^2, [B, N]
    a -= a.mean(axis=1, keepdims=True)        # softmax-invariant centering
    a_hi = a.astype(ml_dtypes.bfloat16).astype(np.float32)
    a_lo = a - a_hi

    in_maps = []
    for core in range(8):
        b, h = divmod(core, 2)
        m_aug = np.empty((K_AUG, N), np.float32)
        m_aug[:CK] = mk[b]
        m_aug[CK] = a_hi[b]
        m_aug[CK + 1] = a_lo[b]

        q_aug = np.empty((K_AUG, HALF), np.float32)
        q_aug[:CK] = 0.25 * qk[b, :, h * HALF : (h + 1) * HALF]
        q_aug[CK] = -0.125
        q_aug[CK + 1] = -0.125

        in_maps.append({"q": q_aug, "m": m_aug})

    res = bass_utils.run_bass_kernel_spmd(
        _get_nc(), in_maps, core_ids=list(range(8))
    )
    _CACHE["last_results"] = res

    out = np.empty((B, N, N), np.float32)
    for core in range(8):
        b, h = divmod(core, 2)
        num = res.results[core]["out_c"].astype(np.float32)   # [2016, 4032]
        den = res.results[core]["sums"].astype(np.float32)    # [126, 16]
        num /= den.T.reshape(HALF, 1)                         # strip-major rows
        out[b, :, h * HALF : (h + 1) * HALF] = num.T
    return out


# revision 14
# speedup vs baseline: 1.0275x; 1.0275x over previous
"""AttentionMemory kernel for Trainium2 (8 NeuronCores, Bass/Tile).

Reference computation (per batch b):
    affinity[n, m] = (2 * mk[:,n]@qk[:,m] - ||mk[:,n]||^2 - ||qk[:,m]||^2) / 8
    out[n, m]      = softmax over n (memory axis)

Softmax over n is invariant to per-column (m) constants, so the
-||qk_m||^2 term and any global constant are dropped.  Logits come from
one augmented fp32r matmul:
    lhsT (stationary) = [0.25 * qk ; -0.125 ; -0.125]   -> [66, Mc]
    rhs  (moving)     = [mk        ; a_hi   ; a_lo  ]   -> [66, N]
    psum[m, n] = 0.25*dot(qk_m, mk_n) - 0.125*a'_n  == logits[m, n]
with a'_n = sum_c mk[c,n]^2 - mean(...) (centering keeps |a'| small so
fp32r rounding of the a-term is negligible; a is additionally split
hi/lo over two rows so the hi part is exact in reduced precision).
fp32r runs at bf16 speed on TRN2 when the moving free dim >= 256
(1 cycle/row); measured logit error ~1.4e-3 -> ~0.14% on the softmax.

The device ships softmax NUMERATORS (exp(logits), bf16) plus per-row
DENOMINATORS (f32 sums); the divide rides the host-side gather pass
that already casts/transposes the result.  bf16 numerator rounding is
a ~0.4% worst-case element error, ~5x under the 2e-2 gate, and halves
HBM store traffic vs f32: per-core 16.25 MB at the modeled 360 GB/s
aggregate DMA = 45 us.

Engine budget per core (cost model): ACT exp is the bottleneck
(1 elem/cycle/partition @ 1.2 GHz): 64512 cols * 0.83 ns + ~185 ns
per-call overhead ~= 60 us busy, gap-free after the first strip.  Row
sums ride DVE tensor_scalar 4x copies (accum_out, 0.26 ns/col) into a
scratch tile, avoiding ACT accum reads (187 ns/call).  PE fp32r ~29 us,
DVE ~22 us, DMA ~50 us — all under ACT.  Output stores depend only on
the exp, so they stream during the strip; the endgame is just the last
strip's sum chain (~3.5 us).

Strip 0 runs in [1,1,2,4]-chunk pieces so the ACT stream starts as
soon as the first 504-column m-chunk lands; the last strip's second
half runs as two 2-chunk pieces to shorten the drain.

Sharding: core c handles batch c//2, query-column half c%2 (softmax is
over the full n axis which each core holds).  Each core writes
out_c[m, n] bf16 + sums[126, 16]; the host divides, casts to f32 and
transposes to the reference [n, m] layout.
"""

import numpy as np

B, CK, H, W = 4, 64, 48, 84
N = H * W            # 4032 memory pixels (softmax axis)
HALF = N // 2        # 2016 query pixels per core
M_STRIP = 126        # output-partition strip size (16 * 126 = 2016)
N_STRIPS = HALF // M_STRIP
K_AUG = CK + 2       # 66: contraction dim incl. the a_hi / a_lo rows

N_CHUNK = 504        # matmul moving free dim (one PSUM bank, 8 pad cols)
N_CHUNKS = N // N_CHUNK  # 8

_CACHE = {}

# Input load schedule: (ring, tensor, col0, width).  Order = program order
# per ring; the single DMA wire serves transfers in ready order.
INPUT_PIECES = [
    ("pool", "q", 0, 252),
    ("sp", "m", 0, 504),
    ("sp", "m", 504, 504),
    ("sp", "m", 1008, 1008),
    ("sp", "m", 2016, 1008),
    ("sp", "m", 3024, 1008),
    ("sp", "q", 252, 1764),
]
# ACT piece widths (in 504-col chunks) per strip
PIECES_FIRST = [1, 1, 2, 2, 2]
PIECES_LAST = [4, 2, 2]
PIECES_MID = [4, 4]
STORE_RING = "pool"      # steady-state store ring
STORE_RINGS_LAST = ["sp", "sp", "sp"]  # last strip, per piece
N_WARM = 12              # PE pstate spin-up matmuls
WARM_MEMSET_RING = "vector"  # engine for the warm-source memset


def _build_nc():
    import concourse.bacc as bacc
    import concourse.mybir as mybir
    import concourse.tile as tile

    f32 = mybir.dt.float32
    f32r = mybir.dt.float32r
    bf16 = mybir.dt.bfloat16
    Exp = mybir.ActivationFunctionType.Exp
    mult = mybir.AluOpType.mult
    add = mybir.AluOpType.add

    nc = bacc.Bacc("TRN2", target_bir_lowering=False, debug=False)

    q_d = nc.dram_tensor("q", [K_AUG, HALF], f32r, kind="ExternalInput")
    m_d = nc.dram_tensor("m", [K_AUG, N], f32r, kind="ExternalInput")
    out_d = nc.dram_tensor("out_c", [HALF, N], bf16, kind="ExternalOutput")
    sums_d = nc.dram_tensor(
        "sums", [M_STRIP, N_STRIPS], f32, kind="ExternalOutput"
    )

    with tile.TileContext(nc) as tc:
        with (
            tc.tile_pool(name="singles", bufs=1) as singles,
            tc.tile_pool(name="psum", bufs=2, space="PSUM") as psum_pool,
            tc.tile_pool(name="exp", bufs=3) as exp_pool,
            tc.tile_pool(name="scratch", bufs=2) as scr_pool,
            tc.tile_pool(name="stats", bufs=4) as stats_pool,
        ):
            # --- inputs, all on the SP ring in exact first-use order (one
            # shared DMA wire; a big transfer issued on another ring would
            # jump ahead of the m chunks and stall the ACT stream) --------
            q_s = singles.tile([K_AUG, HALF], f32r)
            m_s = singles.tile([K_AUG, N], f32r)
            sums_all = singles.tile([M_STRIP, N_STRIPS], f32)
            rings = {
                "sp": nc.sync,
                "act": nc.scalar,
                "pool": nc.gpsimd,
            }
            for ring, tens, c0, w in INPUT_PIECES:
                src_d, dst_s = (q_d, q_s) if tens == "q" else (m_d, m_s)
                rings[ring].dma_start(
                    out=dst_s[:, c0 : c0 + w], in_=src_d[:, c0 : c0 + w]
                )

            # --- prewarm: ACT exp table load + PE pstate spin-up during
            # the input DMAs ---------------------------------------------
            wtab = singles.tile([1, 2], f32)
            nc.vector.memset(wtab, 0.0)
            nc.scalar.activation(wtab[:, 1:2], wtab[:, 0:1], Exp)
            wsrc = singles.tile([K_AUG, 256], bf16)
            getattr(nc, WARM_MEMSET_RING).memset(wsrc, 0.0)
            wps = psum_pool.tile([M_STRIP, 256], f32, tag="ps")
            for _ in range(N_WARM):
                nc.tensor.matmul(
                    wps, wsrc[:, :M_STRIP], wsrc, start=True, stop=True
                )

            for s in range(N_STRIPS):
                m0 = s * M_STRIP
                q_l = q_s[:, m0 : m0 + M_STRIP]

                # piece widths in 504-col chunks; sum to 8 per strip
                if s == 0:
                    pieces = PIECES_FIRST
                elif s == N_STRIPS - 1:
                    pieces = PIECES_LAST
                else:
                    pieces = PIECES_MID

                exp_t = exp_pool.tile([M_STRIP, N], bf16, tag="exp")
                acc = stats_pool.tile([M_STRIP, len(pieces)], f32, tag="acc")

                c = 0
                for pi, w in enumerate(pieces):
                    ps = psum_pool.tile([M_STRIP, 512 * w], f32, tag="ps")
                    for cc in range(w):
                        nc.tensor.matmul(
                            ps[:, cc * 512 : cc * 512 + N_CHUNK],
                            q_l,
                            m_s[:, (c + cc) * N_CHUNK : (c + cc + 1) * N_CHUNK],
                            start=True,
                            stop=True,
                        )
                    # exp(logits) PSUM->SBUF bf16; 3D views skip the pad
                    e0 = c * N_CHUNK
                    e1 = (c + w) * N_CHUNK
                    nc.scalar.activation(
                        exp_t[:, e0:e1].rearrange("p (b c) -> p b c", b=w),
                        ps.rearrange("p (b c) -> p b c", b=w)[:, :, :N_CHUNK],
                        Exp,
                    )
                    # store the numerators as soon as the exp lands
                    # (Pool SWDGE ring in steady state: Pool is otherwise
                    # idle so store dispatch never contends; the drain-
                    # critical last strip uses the lowest-latency ring)
                    if s == N_STRIPS - 1:
                        sring = STORE_RINGS_LAST[pi]
                    else:
                        sring = STORE_RING
                    rings[sring].dma_start(
                        out=out_d[m0 : m0 + M_STRIP, e0:e1], in_=exp_t[:, e0:e1]
                    )
                    # row-sum rides a DVE 4x copy (accum_out) into scratch
                    scr = scr_pool.tile([M_STRIP, 4 * N_CHUNK], bf16, tag="scr")
                    nc.vector.tensor_scalar(
                        out=scr[:, : e1 - e0],
                        in0=exp_t[:, e0:e1],
                        scalar1=1.0,
                        scalar2=None,
                        op0=mult,
                        op1=add,
                        accum_out=acc[:, pi : pi + 1],
                    )
                    c += w

                nc.vector.reduce_sum(
                    sums_all[:, s : s + 1],
                    acc[:, : len(pieces)],
                    axis=mybir.AxisListType.X,
                )

            nc.scalar.dma_start(out=sums_d[:, :], in_=sums_all)

    nc.compile()
    return nc


def _get_nc():
    if "nc" not in _CACHE:
        _CACHE["nc"] = _build_nc()
    return _CACHE["nc"]


def kernel(mk: np.ndarray, qk: np.ndarray) -> np.ndarray:
    import ml_dtypes
    from concourse import bass_utils

    mk = np.asarray(mk, dtype=np.float32).reshape(B, CK, N)
    qk = np.asarray(qk, dtype=np.float32).reshape(B, CK, N)
    a = np.einsum("bcn,bcn->bn", mk, mk)      # sum_c mk^2, [B, N]
    a -= a.mean(axis=1, keepdims=True)        # softmax-invariant centering
    a_hi = a.astype(ml_dtypes.bfloat16).astype(np.float32)
    a_lo = a - a_hi

    in_maps = []
    for core in range(8):
        b, h = divmod(core, 2)
        m_aug = np.empty((K_AUG, N), np.float32)
        m_aug[:CK] = mk[b]
        m_aug[CK] = a_hi[b]
        m_aug[CK + 1] = a_lo[b]

        q_aug = np.empty((K_AUG, HALF), np.float32)
        q_aug[:CK] = 0.25 * qk[b, :, h * HALF : (h + 1) * HALF]
        q_aug[CK] = -0.125
        q_aug[CK + 1] = -0.125

        in_maps.append({"q": q_aug, "m": m_aug})

    res = bass_utils.run_bass_kernel_spmd(
        _get_nc(), in_maps, core_ids=list(range(8))
    )
    _CACHE["last_results"] = res

    out = np.empty((B, N, N), np.float32)
    for core in range(8):
        b, h = divmod(core, 2)
        num = res.results[core]["out_c"].astype(np.float32)   # [2016, 4032]
        den = res.results[core]["sums"].astype(np.float32)    # [126, 16]
        num /= den.T.reshape(HALF, 1)                         # strip-major rows
        out[b, :, h * HALF : (h + 1) * HALF] = num.T
    return out


# revision 17
# speedup vs baseline: 1.0320x; 1.0044x over previous
"""AttentionMemory kernel for Trainium2 (8 NeuronCores, Bass/Tile).

Reference computation (per batch b):
    affinity[n, m] = (2 * mk[:,n]@qk[:,m] - ||mk[:,n]||^2 - ||qk[:,m]||^2) / 8
    out[n, m]      = softmax over n (memory axis)

Softmax over n is invariant to per-column (m) constants, so the
-||qk_m||^2 term and any global constant are dropped.  Logits come from
one augmented fp32r matmul:
    lhsT (stationary) = [0.25 * qk ; -0.125 ; -0.125]   -> [66, Mc]
    rhs  (moving)     = [mk        ; a_hi   ; a_lo  ]   -> [66, N]
    psum[m, n] = 0.25*dot(qk_m, mk_n) - 0.125*a'_n  == logits[m, n]
with a'_n = sum_c mk[c,n]^2 - mean(...) (centering keeps |a'| small so
fp32r rounding of the a-term is negligible; a is additionally split
hi/lo over two rows so the hi part is exact in reduced precision).
fp32r runs at bf16 speed on TRN2 when the moving free dim >= 256
(1 cycle/row); measured logit error ~1.4e-3 -> ~0.14% on the softmax.

The device ships softmax NUMERATORS (exp(logits), bf16) plus per-row
DENOMINATORS (f32 sums); the divide rides the host-side gather pass
that already casts/transposes the result.  bf16 numerator rounding is
a ~0.4% worst-case element error, ~5x under the 2e-2 gate, and halves
HBM store traffic vs f32: per-core 16.25 MB at the modeled 360 GB/s
aggregate DMA = 45 us.

Engine budget per core (cost model): ACT exp is the bottleneck
(1 elem/cycle/partition @ 1.2 GHz): 64512 cols * 0.83 ns + ~185 ns
per-call overhead ~= 60.6 us busy, and the schedule keeps it gap-free
from the first strip to the last.  Row sums ride DVE tensor_scalar 4x
copies (accum_out, 0.26 ns/col) into a scratch tile, avoiding ACT
accum reads (187 ns/call).  PE fp32r ~29 us, DVE ~21 us, DMA wire
~50 us — all under ACT.  Output stores depend only on the exp (not the
sums), so they stream during the strip in 1008-col sub-stores.

Schedule shape (sim-tuned): inputs ride the SP HWDGE ring in first-use
order (505-ns prep cascade == the wire cadence), except the first tiny
q piece which rides the Pool SWDGE ring so both land before the first
matmul; strip 0 runs [1,1,2,2,2]-chunk ACT pieces so exp work covers
the input-arrival window; the last strip runs [4,2,2] with all its
stores + the sums on the lowest-latency rings to shorten the drain.
exp tiles are triple-buffered: a store's completion semaphore (+900 ns)
must release the buffer before ACT laps it.

Sharding: core c handles batch c//2, query-column half c%2 (softmax is
over the full n axis which each core holds).  Each core writes
out_c[m, n] bf16 + sums[126, 16]; the host divides, casts to f32 and
transposes to the reference [n, m] layout.
"""

import numpy as np

B, CK, H, W = 4, 64, 48, 84
N = H * W            # 4032 memory pixels (softmax axis)
HALF = N // 2        # 2016 query pixels per core
M_STRIP = 126        # output-partition strip size (16 * 126 = 2016)
N_STRIPS = HALF // M_STRIP
K_AUG = CK + 2       # 66: contraction dim incl. the a_hi / a_lo rows

N_CHUNK = 504        # matmul moving free dim (one PSUM bank, 8 pad cols)
N_CHUNKS = N // N_CHUNK  # 8

_CACHE = {}

# Input load schedule: (ring, tensor, col0, width).  Order = program order
# per ring; the single DMA wire serves transfers in ready order.
INPUT_PIECES = [
    ("pool", "q", 0, 252),
    ("sp", "m", 0, 504),
    ("sp", "m", 504, 504),
    ("sp", "m", 1008, 1008),
    ("sp", "m", 2016, 1008),
    ("sp", "m", 3024, 1008),
    ("sp", "q", 252, 1764),
]
# ACT piece widths (in 504-col chunks) per strip
PIECES_FIRST = [1, 1, 2, 2, 2]
PIECES_LAST = [4, 2, 2]
PIECES_MID = [4, 4]
STORE_RING = "sp"        # steady-state store ring
STORE_RINGS_LAST = ["sp", "sp", "sp"]  # last strip, per piece
STORE_SPLIT = 2          # sub-stores per 4-chunk piece
N_WARM = 12              # PE pstate spin-up matmuls
WARM_MEMSET_RING = "vector"  # engine for the warm-source memset


def _build_nc():
    import concourse.bacc as bacc
    import concourse.mybir as mybir
    import concourse.tile as tile

    f32 = mybir.dt.float32
    f32r = mybir.dt.float32r
    bf16 = mybir.dt.bfloat16
    Exp = mybir.ActivationFunctionType.Exp
    mult = mybir.AluOpType.mult
    add = mybir.AluOpType.add

    nc = bacc.Bacc("TRN2", target_bir_lowering=False, debug=False)

    q_d = nc.dram_tensor("q", [K_AUG, HALF], f32r, kind="ExternalInput")
    m_d = nc.dram_tensor("m", [K_AUG, N], f32r, kind="ExternalInput")
    out_d = nc.dram_tensor("out_c", [HALF, N], bf16, kind="ExternalOutput")
    sums_d = nc.dram_tensor(
        "sums", [M_STRIP, N_STRIPS], f32, kind="ExternalOutput"
    )

    with tile.TileContext(nc) as tc:
        with (
            tc.tile_pool(name="singles", bufs=1) as singles,
            tc.tile_pool(name="psum", bufs=2, space="PSUM") as psum_pool,
            tc.tile_pool(name="exp", bufs=3) as exp_pool,
            tc.tile_pool(name="scratch", bufs=2) as scr_pool,
            tc.tile_pool(name="stats", bufs=4) as stats_pool,
        ):
            # --- inputs, all on the SP ring in exact first-use order (one
            # shared DMA wire; a big transfer issued on another ring would
            # jump ahead of the m chunks and stall the ACT stream) --------
            q_s = singles.tile([K_AUG, HALF], f32r)
            m_s = singles.tile([K_AUG, N], f32r)
            sums_all = singles.tile([M_STRIP, N_STRIPS], f32)
            rings = {
                "sp": nc.sync,
                "act": nc.scalar,
                "pool": nc.gpsimd,
            }
            for ring, tens, c0, w in INPUT_PIECES:
                src_d, dst_s = (q_d, q_s) if tens == "q" else (m_d, m_s)
                rings[ring].dma_start(
                    out=dst_s[:, c0 : c0 + w], in_=src_d[:, c0 : c0 + w]
                )

            # --- prewarm: ACT exp table load + PE pstate spin-up during
            # the input DMAs ---------------------------------------------
            wtab = singles.tile([1, 2], f32)
            nc.vector.memset(wtab, 0.0)
            nc.scalar.activation(wtab[:, 1:2], wtab[:, 0:1], Exp)
            wsrc = singles.tile([K_AUG, 256], bf16)
            getattr(nc, WARM_MEMSET_RING).memset(wsrc, 0.0)
            wps = psum_pool.tile([M_STRIP, 256], f32, tag="ps")
            for _ in range(N_WARM):
                nc.tensor.matmul(
                    wps, wsrc[:, :M_STRIP], wsrc, start=True, stop=True
                )

            for s in range(N_STRIPS):
                m0 = s * M_STRIP
                q_l = q_s[:, m0 : m0 + M_STRIP]

                # piece widths in 504-col chunks; sum to 8 per strip
                if s == 0:
                    pieces = PIECES_FIRST
                elif s == N_STRIPS - 1:
                    pieces = PIECES_LAST
                else:
                    pieces = PIECES_MID

                exp_t = exp_pool.tile([M_STRIP, N], bf16, tag="exp")
                acc = stats_pool.tile([M_STRIP, len(pieces)], f32, tag="acc")

                c = 0
                for pi, w in enumerate(pieces):
                    ps = psum_pool.tile([M_STRIP, 512 * w], f32, tag="ps")
                    for cc in range(w):
                        nc.tensor.matmul(
                            ps[:, cc * 512 : cc * 512 + N_CHUNK],
                            q_l,
                            m_s[:, (c + cc) * N_CHUNK : (c + cc + 1) * N_CHUNK],
                            start=True,
                            stop=True,
                        )
                    # exp(logits) PSUM->SBUF bf16; 3D views skip the pad
                    e0 = c * N_CHUNK
                    e1 = (c + w) * N_CHUNK
                    nc.scalar.activation(
                        exp_t[:, e0:e1].rearrange("p (b c) -> p b c", b=w),
                        ps.rearrange("p (b c) -> p b c", b=w)[:, :, :N_CHUNK],
                        Exp,
                    )
                    # store the numerators as soon as the exp lands
                    # (Pool SWDGE ring in steady state: Pool is otherwise
                    # idle so store dispatch never contends; the drain-
                    # critical last strip uses the lowest-latency ring)
                    if s == N_STRIPS - 1:
                        sring = STORE_RINGS_LAST[pi]
                    else:
                        sring = STORE_RING
                    nsub = STORE_SPLIT if w == 4 else 1
                    sw = (e1 - e0) // nsub
                    for k in range(nsub):
                        rings[sring].dma_start(
                            out=out_d[m0 : m0 + M_STRIP, e0 + k * sw : e0 + (k + 1) * sw],
                            in_=exp_t[:, e0 + k * sw : e0 + (k + 1) * sw],
                        )
                    # row-sum rides a DVE 4x copy (accum_out) into scratch
                    scr = scr_pool.tile([M_STRIP, 4 * N_CHUNK], bf16, tag="scr")
                    nc.vector.tensor_scalar(
                        out=scr[:, : e1 - e0],
                        in0=exp_t[:, e0:e1],
                        scalar1=1.0,
                        scalar2=None,
                        op0=mult,
                        op1=add,
                        accum_out=acc[:, pi : pi + 1],
                    )
                    c += w

                nc.vector.reduce_sum(
                    sums_all[:, s : s + 1],
                    acc[:, : len(pieces)],
                    axis=mybir.AxisListType.X,
                )

            nc.scalar.dma_start(out=sums_d[:, :], in_=sums_all)

    nc.compile()
    return nc


def _get_nc():
    if "nc" not in _CACHE:
        _CACHE["nc"] = _build_nc()
    return _CACHE["nc"]


def kernel(mk: np.ndarray, qk: np.ndarray) -> np.ndarray:
    import ml_dtypes
    from concourse import bass_utils

    mk = np.asarray(mk, dtype=np.float32).reshape(B, CK, N)
    qk = np.asarray(qk, dtype=np.float32).reshape(B, CK, N)
    a = np.einsum("bcn,bcn->bn", mk, mk)      # sum_c mk^2, [B, N]
    a -= a.mean(axis=1, keepdims=True)        # softmax-invariant centering
    a_hi = a.astype(ml_dtypes.bfloat16).astype(np.float32)
    a_lo = a - a_hi

    in_maps = []
    for core in range(8):
        b, h = divmod(core, 2)
        m_aug = np.empty((K_AUG, N), np.float32)
        m_aug[:CK] = mk[b]
        m_aug[CK] = a_hi[b]
        m_aug[CK + 1] = a_lo[b]

        q_aug = np.empty((K_AUG, HALF), np.float32)
        q_aug[:CK] = 0.25 * qk[b, :, h * HALF : (h + 1) * HALF]
        q_aug[CK] = -0.125
        q_aug[CK + 1] = -0.125

        in_maps.append({"q": q_aug, "m": m_aug})

    res = bass_utils.run_bass_kernel_spmd(
        _get_nc(), in_maps, core_ids=list(range(8))
    )
    _CACHE["last_results"] = res

    out = np.empty((B, N, N), np.float32)
    for core in range(8):
        b, h = divmod(core, 2)
        num = res.results[core]["out_c"].astype(np.float32)   # [2016, 4032]
        den = res.results[core]["sums"].astype(np.float32)    # [126, 16]
        num /= den.T.reshape(HALF, 1)                         # strip-major rows
        out[b, :, h * HALF : (h + 1) * HALF] = num.T
    return out


# revision 18
# speedup vs baseline: 1.0341x; 1.0021x over previous
"""AttentionMemory kernel for Trainium2 (8 NeuronCores, Bass/Tile).

Reference computation (per batch b):
    affinity[n, m] = (2 * mk[:,n]@qk[:,m] - ||mk[:,n]||^2 - ||qk[:,m]||^2) / 8
    out[n, m]      = softmax over n (memory axis)

Softmax over n is invariant to per-column (m) constants, so the
-||qk_m||^2 term and any global constant are dropped.  Logits come from
one augmented fp32r matmul:
    lhsT (stationary) = [0.25 * qk ; -0.125 ; -0.125]   -> [66, Mc]
    rhs  (moving)     = [mk        ; a_hi   ; a_lo  ]   -> [66, N]
    psum[m, n] = 0.25*dot(qk_m, mk_n) - 0.125*a'_n  == logits[m, n]
with a'_n = sum_c mk[c,n]^2 - mean(...) (centering keeps |a'| small so
fp32r rounding of the a-term is negligible; a is additionally split
hi/lo over two rows so the hi part is exact in reduced precision).
fp32r runs at bf16 speed on TRN2 when the moving free dim >= 256
(1 cycle/row); measured logit error ~1.4e-3 -> ~0.14% on the softmax.

The device ships softmax NUMERATORS (exp(logits), bf16) plus per-row
DENOMINATORS (f32 sums); the divide rides the host-side gather pass
that already casts/transposes the result.  bf16 numerator rounding is
a ~0.4% worst-case element error, ~5x under the 2e-2 gate, and halves
HBM store traffic vs f32: per-core 16.25 MB at the modeled 360 GB/s
aggregate DMA = 45 us.

Engine budget per core (cost model): ACT exp is the bottleneck
(1 elem/cycle/partition @ 1.2 GHz): 64512 cols * 0.83 ns + ~185 ns
per-call overhead ~= 60.6 us busy, and the schedule keeps it gap-free
from the first strip to the last.  Row sums ride DVE tensor_scalar 4x
copies (accum_out, 0.26 ns/col) into a scratch tile, avoiding ACT
accum reads (187 ns/call).  PE fp32r ~29 us, DVE ~21 us, DMA wire
~50 us — all under ACT.  Output stores depend only on the exp (not the
sums), so they stream during the strip in 1008-col sub-stores.

Schedule shape (sim-tuned): inputs ride the SP HWDGE ring in first-use
order (505-ns prep cascade == the wire cadence), except the first tiny
q piece which rides the Pool SWDGE ring so both land before the first
matmul; strip 0 runs [1,1,2,2,2]-chunk ACT pieces so exp work covers
the input-arrival window; the last strip runs [4,2,2] with all its
stores + the sums on the lowest-latency rings to shorten the drain.
exp tiles are triple-buffered: a store's completion semaphore (+900 ns)
must release the buffer before ACT laps it.

Sharding: core c handles batch c//2, query-column half c%2 (softmax is
over the full n axis which each core holds).  Each core writes
out_c[m, n] bf16 + sums[126, 16]; the host divides, casts to f32 and
transposes to the reference [n, m] layout.
"""

import numpy as np

B, CK, H, W = 4, 64, 48, 84
N = H * W            # 4032 memory pixels (softmax axis)
HALF = N // 2        # 2016 query pixels per core
M_STRIP = 126        # output-partition strip size (16 * 126 = 2016)
N_STRIPS = HALF // M_STRIP
K_AUG = CK + 2       # 66: contraction dim incl. the a_hi / a_lo rows

N_CHUNK = 504        # matmul moving free dim (one PSUM bank, 8 pad cols)
N_CHUNKS = N // N_CHUNK  # 8

_CACHE = {}

# Input load schedule over the packed qm tensor: (ring, col0, width).
# qm columns: [0:252] = q[0:252], [252:4284] = m, [4284:6048] = q[252:].
# Order = program order per ring; the single DMA wire serves transfers in
# ready order.  The first piece spans q-strip0/1 AND m-chunk0 so one
# HWDGE prep (625 ns) delivers everything the first matmul needs.
QM_COLS = 6048
INPUT_PIECES = [
    ("sp", 0, 756),
    ("pool", 756, 504),
    ("sp", 1260, 1008),
    ("sp", 2268, 1008),
    ("sp", 3276, 1008),
    ("sp", 4284, 1764),
]
# ACT piece widths (in 504-col chunks) per strip
PIECES_FIRST = [1, 1, 2, 2, 2]
PIECES_LAST = [4, 2, 2]
PIECES_MID = [4, 4]
STORE_RING = "sp"        # steady-state store ring
STORE_RINGS_LAST = ["sp", "sp", "sp"]  # last strip, per piece
STORE_SPLIT = 2          # sub-stores per 4-chunk piece
N_WARM = 10              # PE pstate spin-up matmuls
WARM_MEMSET_RING = "vector"  # engine for the warm-source memset


def _build_nc():
    import concourse.bacc as bacc
    import concourse.mybir as mybir
    import concourse.tile as tile

    f32 = mybir.dt.float32
    f32r = mybir.dt.float32r
    bf16 = mybir.dt.bfloat16
    Exp = mybir.ActivationFunctionType.Exp
    mult = mybir.AluOpType.mult
    add = mybir.AluOpType.add

    nc = bacc.Bacc("TRN2", target_bir_lowering=False, debug=False)

    qm_d = nc.dram_tensor("qm", [K_AUG, QM_COLS], f32r, kind="ExternalInput")
    out_d = nc.dram_tensor("out_c", [HALF, N], bf16, kind="ExternalOutput")
    sums_d = nc.dram_tensor(
        "sums", [M_STRIP, N_STRIPS], f32, kind="ExternalOutput"
    )

    with tile.TileContext(nc) as tc:
        with (
            tc.tile_pool(name="singles", bufs=1) as singles,
            tc.tile_pool(name="psum", bufs=2, space="PSUM") as psum_pool,
            tc.tile_pool(name="exp", bufs=3) as exp_pool,
            tc.tile_pool(name="scratch", bufs=2) as scr_pool,
            tc.tile_pool(name="stats", bufs=4) as stats_pool,
        ):
            # --- inputs, in exact first-use order (one shared DMA wire; a
            # big transfer issued out of order would jump ahead of the m
            # chunks and stall the ACT stream) ----------------------------
            qm_s = singles.tile([K_AUG, QM_COLS], f32r)
            sums_all = singles.tile([M_STRIP, N_STRIPS], f32)
            rings = {
                "sp": nc.sync,
                "act": nc.scalar,
                "pool": nc.gpsimd,
            }
            for ring, c0, w in INPUT_PIECES:
                rings[ring].dma_start(
                    out=qm_s[:, c0 : c0 + w], in_=qm_d[:, c0 : c0 + w]
                )

            def q_cols(c0, w):  # q columns c0:c0+w as packed-qm slice
                assert c0 + w <= 252 or c0 >= 252
                off = c0 if c0 + w <= 252 else 4284 - 252 + c0
                return qm_s[:, off : off + w]

            def m_cols(c0, w):
                return qm_s[:, 252 + c0 : 252 + c0 + w]

            # --- prewarm: ACT exp table load + PE pstate spin-up during
            # the input DMAs ---------------------------------------------
            wtab = singles.tile([1, 2], f32)
            nc.vector.memset(wtab, 0.0)
            nc.scalar.activation(wtab[:, 1:2], wtab[:, 0:1], Exp)
            wsrc = singles.tile([K_AUG, 256], bf16)
            getattr(nc, WARM_MEMSET_RING).memset(wsrc, 0.0)
            wps = psum_pool.tile([M_STRIP, 256], f32, tag="ps")
            for _ in range(N_WARM):
                nc.tensor.matmul(
                    wps, wsrc[:, :M_STRIP], wsrc, start=True, stop=True
                )

            for s in range(N_STRIPS):
                m0 = s * M_STRIP
                q_l = q_cols(m0, M_STRIP)

                # piece widths in 504-col chunks; sum to 8 per strip
                if s == 0:
                    pieces = PIECES_FIRST
                elif s == N_STRIPS - 1:
                    pieces = PIECES_LAST
                else:
                    pieces = PIECES_MID

                exp_t = exp_pool.tile([M_STRIP, N], bf16, tag="exp")
                acc = stats_pool.tile([M_STRIP, len(pieces)], f32, tag="acc")

                c = 0
                for pi, w in enumerate(pieces):
                    ps = psum_pool.tile([M_STRIP, 512 * w], f32, tag="ps")
                    for cc in range(w):
                        nc.tensor.matmul(
                            ps[:, cc * 512 : cc * 512 + N_CHUNK],
                            q_l,
                            m_cols((c + cc) * N_CHUNK, N_CHUNK),
                            start=True,
                            stop=True,
                        )
                    # exp(logits) PSUM->SBUF bf16; 3D views skip the pad
                    e0 = c * N_CHUNK
                    e1 = (c + w) * N_CHUNK
                    nc.scalar.activation(
                        exp_t[:, e0:e1].rearrange("p (b c) -> p b c", b=w),
                        ps.rearrange("p (b c) -> p b c", b=w)[:, :, :N_CHUNK],
                        Exp,
                    )
                    # store the numerators as soon as the exp lands
                    # (Pool SWDGE ring in steady state: Pool is otherwise
                    # idle so store dispatch never contends; the drain-
                    # critical last strip uses the lowest-latency ring)
                    if s == N_STRIPS - 1:
                        sring = STORE_RINGS_LAST[pi]
                    else:
                        sring = STORE_RING
                    nsub = STORE_SPLIT if w == 4 else 1
                    sw = (e1 - e0) // nsub
                    for k in range(nsub):
                        rings[sring].dma_start(
                            out=out_d[m0 : m0 + M_STRIP, e0 + k * sw : e0 + (k + 1) * sw],
                            in_=exp_t[:, e0 + k * sw : e0 + (k + 1) * sw],
                        )
                    # row-sum rides a DVE 4x copy (accum_out) into scratch
                    scr = scr_pool.tile([M_STRIP, 4 * N_CHUNK], bf16, tag="scr")
                    nc.vector.tensor_scalar(
                        out=scr[:, : e1 - e0],
                        in0=exp_t[:, e0:e1],
                        scalar1=1.0,
                        scalar2=None,
                        op0=mult,
                        op1=add,
                        accum_out=acc[:, pi : pi + 1],
                    )
                    c += w

                nc.vector.reduce_sum(
                    sums_all[:, s : s + 1],
                    acc[:, : len(pieces)],
                    axis=mybir.AxisListType.X,
                )

            nc.scalar.dma_start(out=sums_d[:, :], in_=sums_all)

    nc.compile()
    return nc


def _get_nc():
    if "nc" not in _CACHE:
        _CACHE["nc"] = _build_nc()
    return _CACHE["nc"]


def kernel(mk: np.ndarray, qk: np.ndarray) -> np.ndarray:
    import ml_dtypes
    from concourse import bass_utils

    mk = np.asarray(mk, dtype=np.float32).reshape(B, CK, N)
    qk = np.asarray(qk, dtype=np.float32).reshape(B, CK, N)
    a = np.einsum("bcn,bcn->bn", mk, mk)      # sum_c mk^2, [B, N]
    a -= a.mean(axis=1, keepdims=True)        # softmax-invariant centering
    a_hi = a.astype(ml_dtypes.bfloat16).astype(np.float32)
    a_lo = a - a_hi

    in_maps = []
    for core in range(8):
        b, h = divmod(core, 2)
        m_aug = np.empty((K_AUG, N), np.float32)
        m_aug[:CK] = mk[b]
        m_aug[CK] = a_hi[b]
        m_aug[CK + 1] = a_lo[b]

        q_aug = np.empty((K_AUG, HALF), np.float32)
        q_aug[:CK] = 0.25 * qk[b, :, h * HALF : (h + 1) * HALF]
        q_aug[CK] = -0.125
        q_aug[CK + 1] = -0.125

        qm = np.empty((K_AUG, 252 + N + HALF - 252), np.float32)
        qm[:, :252] = q_aug[:, :252]
        qm[:, 252 : 252 + N] = m_aug
        qm[:, 252 + N :] = q_aug[:, 252:]
        in_maps.append({"qm": qm})

    res = bass_utils.run_bass_kernel_spmd(
        _get_nc(), in_maps, core_ids=list(range(8))
    )
    _CACHE["last_results"] = res

    out = np.empty((B, N, N), np.float32)
    for core in range(8):
        b, h = divmod(core, 2)
        num = res.results[core]["out_c"].astype(np.float32)   # [2016, 4032]
        den = res.results[core]["sums"].astype(np.float32)    # [126, 16]
        num /= den.T.reshape(HALF, 1)                         # strip-major rows
        out[b, :, h * HALF : (h + 1) * HALF] = num.T
    return out


# revision 19
# speedup vs baseline: 1.0349x; 1.0007x over previous
"""AttentionMemory kernel for Trainium2 (8 NeuronCores, Bass/Tile).

Reference computation (per batch b):
    affinity[n, m] = (2 * mk[:,n]@qk[:,m] - ||mk[:,n]||^2 - ||qk[:,m]||^2) / 8
    out[n, m]      = softmax over n (memory axis)

Softmax over n is invariant to per-column (m) constants, so the
-||qk_m||^2 term and any global constant are dropped.  Logits come from
one augmented fp32r matmul:
    lhsT (stationary) = [0.25 * qk ; -0.125 ; -0.125]   -> [66, Mc]
    rhs  (moving)     = [mk        ; a_hi   ; a_lo  ]   -> [66, N]
    psum[m, n] = 0.25*dot(qk_m, mk_n) - 0.125*a'_n  == logits[m, n]
with a'_n = sum_c mk[c,n]^2 - mean(...) (centering keeps |a'| small so
fp32r rounding of the a-term is negligible; a is additionally split
hi/lo over two rows so the hi part is exact in reduced precision).
fp32r runs at bf16 speed on TRN2 when the moving free dim >= 256
(1 cycle/row); measured logit error ~1.4e-3 -> ~0.14% on the softmax.

The device ships softmax NUMERATORS (exp(logits), bf16) plus per-row
DENOMINATORS (f32 sums); the divide rides the host-side gather pass
that already casts/transposes the result.  bf16 numerator rounding is
a ~0.4% worst-case element error, ~5x under the 2e-2 gate, and halves
HBM store traffic vs f32: per-core 16.25 MB at the modeled 360 GB/s
aggregate DMA = 45 us.

Engine budget per core (cost model): ACT exp is the bottleneck
(1 elem/cycle/partition @ 1.2 GHz): 64512 cols * 0.83 ns + ~185 ns
per-call overhead ~= 60.6 us busy, and the schedule keeps it gap-free
from the first strip to the last.  Row sums ride DVE tensor_scalar 4x
copies (accum_out, 0.26 ns/col) into a scratch tile, avoiding ACT
accum reads (187 ns/call).  PE fp32r ~29 us, DVE ~21 us, DMA wire
~50 us — all under ACT.  Output stores depend only on the exp (not the
sums), so they stream during the strip in 1008-col sub-stores.

Schedule shape (sim-tuned): inputs ride the SP HWDGE ring in first-use
order (505-ns prep cascade == the wire cadence), except the first tiny
q piece which rides the Pool SWDGE ring so both land before the first
matmul; strip 0 runs [1,1,2,2,2]-chunk ACT pieces so exp work covers
the input-arrival window; the last strip runs [4,2,2] with all its
stores + the sums on the lowest-latency rings to shorten the drain.
exp tiles are triple-buffered: a store's completion semaphore (+900 ns)
must release the buffer before ACT laps it.

Sharding: core c handles batch c//2, query-column half c%2 (softmax is
over the full n axis which each core holds).  Each core writes
out_c[m, n] bf16 + sums[126, 16]; the host divides, casts to f32 and
transposes to the reference [n, m] layout.
"""

import numpy as np

B, CK, H, W = 4, 64, 48, 84
N = H * W            # 4032 memory pixels (softmax axis)
HALF = N // 2        # 2016 query pixels per core
M_STRIP = 126        # output-partition strip size (16 * 126 = 2016)
N_STRIPS = HALF // M_STRIP
K_AUG = CK + 2       # 66: contraction dim incl. the a_hi / a_lo rows

N_CHUNK = 504        # matmul moving free dim (one PSUM bank, 8 pad cols)
N_CHUNKS = N // N_CHUNK  # 8

_CACHE = {}

# Input load schedule over the packed qm tensor: (ring, col0, width).
# qm columns: [0:252] = q[0:252], [252:4284] = m, [4284:6048] = q[252:].
# Order = program order per ring; the single DMA wire serves transfers in
# ready order.  The first piece spans q-strip0/1 AND m-chunk0 so one
# HWDGE prep (625 ns) delivers everything the first matmul needs.
QM_COLS = 6048
INPUT_PIECES = [
    ("sp", 0, 756),
    ("pool", 756, 504),
    ("sp", 1260, 504),
    ("sp", 1764, 504),
    ("sp", 2268, 1008),
    ("sp", 3276, 1008),
    ("sp", 4284, 1764),
]
# ACT piece widths (in 504-col chunks) per strip
PIECES_FIRST = [1, 1, 2, 2, 2]
PIECES_LAST = [4, 2, 2]
PIECES_MID = [4, 4]
STORE_RING = "sp"        # steady-state store ring
STORE_RINGS_LAST = ["sp", "sp", "sp"]  # last strip, per piece
STORE_SPLIT = 2          # sub-stores per 4-chunk piece
N_WARM = 10              # PE pstate spin-up matmuls
WARM_MEMSET_RING = "vector"  # engine for the warm-source memset


def _build_nc():
    import concourse.bacc as bacc
    import concourse.mybir as mybir
    import concourse.tile as tile

    f32 = mybir.dt.float32
    f32r = mybir.dt.float32r
    bf16 = mybir.dt.bfloat16
    Exp = mybir.ActivationFunctionType.Exp
    mult = mybir.AluOpType.mult
    add = mybir.AluOpType.add

    nc = bacc.Bacc("TRN2", target_bir_lowering=False, debug=False)

    qm_d = nc.dram_tensor("qm", [K_AUG, QM_COLS], f32r, kind="ExternalInput")
    out_d = nc.dram_tensor("out_c", [HALF, N], bf16, kind="ExternalOutput")
    sums_d = nc.dram_tensor(
        "sums", [M_STRIP, N_STRIPS], f32, kind="ExternalOutput"
    )

    with tile.TileContext(nc) as tc:
        with (
            tc.tile_pool(name="singles", bufs=1) as singles,
            tc.tile_pool(name="psum", bufs=2, space="PSUM") as psum_pool,
            tc.tile_pool(name="exp", bufs=3) as exp_pool,
            tc.tile_pool(name="scratch", bufs=2) as scr_pool,
            tc.tile_pool(name="stats", bufs=4) as stats_pool,
        ):
            # --- inputs, in exact first-use order (one shared DMA wire; a
            # big transfer issued out of order would jump ahead of the m
            # chunks and stall the ACT stream) ----------------------------
            qm_s = singles.tile([K_AUG, QM_COLS], f32r)
            sums_all = singles.tile([M_STRIP, N_STRIPS], f32)
            rings = {
                "sp": nc.sync,
                "act": nc.scalar,
                "pool": nc.gpsimd,
            }
            for ring, c0, w in INPUT_PIECES:
                rings[ring].dma_start(
                    out=qm_s[:, c0 : c0 + w], in_=qm_d[:, c0 : c0 + w]
                )

            def q_cols(c0, w):  # q columns c0:c0+w as packed-qm slice
                assert c0 + w <= 252 or c0 >= 252
                off = c0 if c0 + w <= 252 else 4284 - 252 + c0
                return qm_s[:, off : off + w]

            def m_cols(c0, w):
                return qm_s[:, 252 + c0 : 252 + c0 + w]

            # --- prewarm: ACT exp table load + PE pstate spin-up during
            # the input DMAs ---------------------------------------------
            wtab = singles.tile([1, 2], f32)
            nc.vector.memset(wtab, 0.0)
            nc.scalar.activation(wtab[:, 1:2], wtab[:, 0:1], Exp)
            wsrc = singles.tile([K_AUG, 256], bf16)
            getattr(nc, WARM_MEMSET_RING).memset(wsrc, 0.0)
            wps = psum_pool.tile([M_STRIP, 256], f32, tag="ps")
            for _ in range(N_WARM):
                nc.tensor.matmul(
                    wps, wsrc[:, :M_STRIP], wsrc, start=True, stop=True
                )

            for s in range(N_STRIPS):
                m0 = s * M_STRIP
                q_l = q_cols(m0, M_STRIP)

                # piece widths in 504-col chunks; sum to 8 per strip
                if s == 0:
                    pieces = PIECES_FIRST
                elif s == N_STRIPS - 1:
                    pieces = PIECES_LAST
                else:
                    pieces = PIECES_MID

                exp_t = exp_pool.tile([M_STRIP, N], bf16, tag="exp")
                acc = stats_pool.tile([M_STRIP, len(pieces)], f32, tag="acc")

                c = 0
                for pi, w in enumerate(pieces):
                    ps = psum_pool.tile([M_STRIP, 512 * w], f32, tag="ps")
                    for cc in range(w):
                        nc.tensor.matmul(
                            ps[:, cc * 512 : cc * 512 + N_CHUNK],
                            q_l,
                            m_cols((c + cc) * N_CHUNK, N_CHUNK),
                            start=True,
                            stop=True,
                        )
                    # exp(logits) PSUM->SBUF bf16; 3D views skip the pad
                    e0 = c * N_CHUNK
                    e1 = (c + w) * N_CHUNK
                    nc.scalar.activation(
                        exp_t[:, e0:e1].rearrange("p (b c) -> p b c", b=w),
                        ps.rearrange("p (b c) -> p b c", b=w)[:, :, :N_CHUNK],
                        Exp,
                    )
                    # store the numerators as soon as the exp lands
                    # (Pool SWDGE ring in steady state: Pool is otherwise
                    # idle so store dispatch never contends; the drain-
                    # critical last strip uses the lowest-latency ring)
                    if s == N_STRIPS - 1:
                        sring = STORE_RINGS_LAST[pi]
                    else:
                        sring = STORE_RING
                    nsub = STORE_SPLIT if w == 4 else 1
                    sw = (e1 - e0) // nsub
                    for k in range(nsub):
                        rings[sring].dma_start(
                            out=out_d[m0 : m0 + M_STRIP, e0 + k * sw : e0 + (k + 1) * sw],
                            in_=exp_t[:, e0 + k * sw : e0 + (k + 1) * sw],
                        )
                    # row-sum rides a DVE 4x copy (accum_out) into scratch
                    scr = scr_pool.tile([M_STRIP, 4 * N_CHUNK], bf16, tag="scr")
                    nc.vector.tensor_scalar(
                        out=scr[:, : e1 - e0],
                        in0=exp_t[:, e0:e1],
                        scalar1=1.0,
                        scalar2=None,
                        op0=mult,
                        op1=add,
                        accum_out=acc[:, pi : pi + 1],
                    )
                    c += w

                nc.vector.reduce_sum(
                    sums_all[:, s : s + 1],
                    acc[:, : len(pieces)],
                    axis=mybir.AxisListType.X,
                )

            nc.scalar.dma_start(out=sums_d[:, :], in_=sums_all)

    nc.compile()
    return nc


def _get_nc():
    if "nc" not in _CACHE:
        _CACHE["nc"] = _build_nc()
    return _CACHE["nc"]


def kernel(mk: np.ndarray, qk: np.ndarray) -> np.ndarray:
    import ml_dtypes
    from concourse import bass_utils

    mk = np.asarray(mk, dtype=np.float32).reshape(B, CK, N)
    qk = np.asarray(qk, dtype=np.float32).reshape(B, CK, N)
    a = np.einsum("bcn,bcn->bn", mk, mk)      # sum_c mk^2, [B, N]
    a -= a.mean(axis=1, keepdims=True)        # softmax-invariant centering
    a_hi = a.astype(ml_dtypes.bfloat16).astype(np.float32)
    a_lo = a - a_hi

    in_maps = []
    for core in range(8):
        b, h = divmod(core, 2)
        m_aug = np.empty((K_AUG, N), np.float32)
        m_aug[:CK] = mk[b]
        m_aug[CK] = a_hi[b]
        m_aug[CK + 1] = a_lo[b]

        q_aug = np.empty((K_AUG, HALF), np.float32)
        q_aug[:CK] = 0.25 * qk[b, :, h * HALF : (h + 1) * HALF]
        q_aug[CK] = -0.125
        q_aug[CK + 1] = -0.125

        qm = np.empty((K_AUG, 252 + N + HALF - 252), np.float32)
        qm[:, :252] = q_aug[:, :252]
        qm[:, 252 : 252 + N] = m_aug
        qm[:, 252 + N :] = q_aug[:, 252:]
        in_maps.append({"qm": qm})

    res = bass_utils.run_bass_kernel_spmd(
        _get_nc(), in_maps, core_ids=list(range(8))
    )
    _CACHE["last_results"] = res

    out = np.empty((B, N, N), np.float32)
    for core in range(8):
        b, h = divmod(core, 2)
        num = res.results[core]["out_c"].astype(np.float32)   # [2016, 4032]
        den = res.results[core]["sums"].astype(np.float32)    # [126, 16]
        num /= den.T.reshape(HALF, 1)                         # strip-major rows
        out[b, :, h * HALF : (h + 1) * HALF] = num.T
    return out
